# revision 3
# baseline (speedup 1.0000x reference)
"""Multi-head attention (B=4, T=2048, D=1024, H=16) on 8 TRN2 NeuronCores.

Sharding: core c -> (batch b = c//2, head-group g = c%2 of 8 heads).
Each core computes qkv projection for its batch restricted to its 8 heads,
full attention for those heads, and a partial output projection
(ctx_local @ Wout[rows of its heads]).  Host sums the two partials per batch.

Per-core kernel layout:
  phase 1: stream x^T tiles, round to fp32r, compute qT/kT (bf16, [dh, tok])
           and v (bf16, [tok, dh] with a ones column for sumexp).
  phase 2: per (head, quarter-q): S^T chunks = kT.T @ qT on PE (bf16),
           P = exp(0.125*S) on ACT (PSUM->SBUF bf16),
           ctx^T[d,q] (+ sumexp row) = sum_k [v|1].T @ P on PE,
           normalize via DVE reciprocal + gpsimd partition broadcast.
  phase 3: out = ctx^T.T @ Wout (fp32r), DMA partial [2048,1024] out.
"""

import sys
import types
import numpy as np
from contextlib import ExitStack

import concourse.bass as bass
import concourse.bacc as bacc
import concourse.tile as tile
from concourse import mybir
from concourse.bass_utils import run_bass_kernel_spmd

FP32 = mybir.dt.float32
F32R = mybir.dt.float32r
BF16 = mybir.dt.bfloat16
EXP = mybir.ActivationFunctionType.Exp

D = 1024
T = 2048
HPC = 8          # heads per core
FC = 8           # feature chunks of 128 (contraction for projections)
TS = 4           # token spans of 512
TCN = 16         # token chunks of 128
KC = 16          # k chunks of 128
QQ = 4           # query quarters of 512
EXP_GROUPS = [(0, 3), (3, 3), (6, 3), (9, 3), (12, 3), (15, 1)]


def _body(ctx, nc, tc, xt_d, wq_d, wk_d, wv_d, wo_d, out_d):
    persist = ctx.enter_context(tc.tile_pool(name="persist", bufs=1))
    qT = persist.tile([128, 4, T], BF16, tag="qT")
    kT = persist.tile([128, 4, T], BF16, tag="kT")
    v_sb = persist.tile([128, TCN, HPC, 65], BF16, tag="v")
    ctx_sb = persist.tile([128, 4, T], F32R, tag="ctx")

    nc.vector.memset(v_sb[:, :, :, 64:65], 1.0)

    # ---------------- phase 1: projections ----------------
    with tc.tile_pool(name="wraw", bufs=2) as wraw, \
         tc.tile_pool(name="wpool", bufs=1) as wpool, \
         tc.tile_pool(name="xraw", bufs=4) as xraw, \
         tc.tile_pool(name="xr", bufs=10) as xrp, \
         tc.tile_pool(name="ps1", bufs=8, space="PSUM") as ps1:

        w_sbs = {}
        for wname, wd in (("wq", wq_d), ("wk", wk_d), ("wv", wv_d)):
            w_sb = wpool.tile([128, FC, 512], F32R, tag=wname)
            w_sbs[wname] = w_sb
            for fc in range(FC):
                raw = wraw.tile([128, 512], FP32, tag="wraw")
                nc.sync.dma_start(out=raw[:], in_=wd[fc * 128:(fc + 1) * 128, :])
                nc.vector.tensor_copy(out=w_sb[:, fc, :], in_=raw[:])
        wq_sb, wk_sb, wv_sb = w_sbs["wq"], w_sbs["wk"], w_sbs["wv"]

        for ts in range(TS):
            xts = []
            for fc in range(FC):
                raw = xraw.tile([128, 512], FP32, tag="xraw")
                nc.sync.dma_start(
                    out=raw[:],
                    in_=xt_d[fc * 128:(fc + 1) * 128, ts * 512:(ts + 1) * 512])
                xr = xrp.tile([128, 512], F32R, tag="xr")
                nc.vector.tensor_copy(out=xr[:], in_=raw[:])
                xts.append(xr)
            # qT / kT: out [col 128, tok 512]
            for cc8 in range(8):
                w_sb = wq_sb if cc8 < 4 else wk_sb
                dst = qT if cc8 < 4 else kT
                ccl = cc8 % 4
                ps = ps1.tile([128, 512], FP32, tag="ps1")
                for fc in range(FC):
                    nc.tensor.matmul(
                        ps[:],
                        lhsT=w_sb[:, fc, ccl * 128:(ccl + 1) * 128],
                        rhs=xts[fc][:],
                        start=(fc == 0), stop=(fc == FC - 1))
                nc.vector.tensor_copy(
                    out=dst[:, ccl, ts * 512:(ts + 1) * 512], in_=ps[:])
            # v: out [tok 128, col 512]
            for tc4 in range(4):
                tcg = ts * 4 + tc4
                psv = ps1.tile([128, 512], FP32, tag="ps1")
                for fc in range(FC):
                    nc.tensor.matmul(
                        psv[:],
                        lhsT=xts[fc][:, tc4 * 128:(tc4 + 1) * 128],
                        rhs=wv_sb[:, fc, :],
                        start=(fc == 0), stop=(fc == FC - 1))
                for hh in range(HPC):
                    nc.vector.tensor_copy(
                        out=v_sb[:, tcg, hh, 0:64],
                        in_=psv[:, hh * 64:(hh + 1) * 64])

    # ---------------- phase 2: attention ----------------
    with tc.tile_pool(name="P", bufs=2) as ppool, \
         tc.tile_pool(name="spsum", bufs=2, space="PSUM") as spsum, \
         tc.tile_pool(name="cpsum", bufs=2, space="PSUM") as cpsum, \
         tc.tile_pool(name="rpool", bufs=2) as rpool:

        for hh in range(HPC):
            hb = (hh % 2) * 64
            hc = hh // 2
            for qq in range(QQ):
                qsl = slice(qq * 512, (qq + 1) * 512)
                P = ppool.tile([128, KC, 512], BF16, tag="P")
                ctxp = cpsum.tile([65, 512], FP32, tag="ctx")
                for (k0, nk) in EXP_GROUPS:
                    sps = spsum.tile([128, 3, 512], FP32, tag="S")
                    for i in range(nk):
                        kc = k0 + i
                        nc.tensor.matmul(
                            sps[:, i, :],
                            lhsT=kT[hb:hb + 64, hc, kc * 128:(kc + 1) * 128],
                            rhs=qT[hb:hb + 64, hc, qsl],
                            start=True, stop=True)
                    nc.scalar.activation(
                        out=P[:, k0:k0 + nk, :], in_=sps[:, 0:nk, :],
                        func=EXP, scale=0.125)
                    for i in range(nk):
                        kc = k0 + i
                        nc.tensor.matmul(
                            ctxp[:],
                            lhsT=v_sb[:, kc, hh, :],
                            rhs=P[:, kc, :],
                            start=(kc == 0), stop=(kc == KC - 1))
                # normalize: ctx_sb[hb:hb+64, hc, qsl] = ctxp[0:64] / ctxp[64]
                rtmp = rpool.tile([1, 512], FP32, tag="rtmp")
                nc.vector.tensor_copy(out=rtmp[:], in_=ctxp[64:65, :])
                rt = rpool.tile([1, 512], FP32, tag="rt")
                nc.vector.reciprocal_approx_fast(out=rt[:], in_=rtmp[:])
                rb = rpool.tile([64, 512], FP32, tag="rb")
                nc.gpsimd.partition_broadcast(rb[:], rt[0:1, :], channels=64)
                nc.vector.tensor_mul(
                    ctx_sb[hb:hb + 64, hc, qsl], ctxp[0:64, :], rb[:])

    # ---------------- phase 3: output projection ----------------
    with tc.tile_pool(name="woraw", bufs=2) as woraw, \
         tc.tile_pool(name="wo", bufs=1) as wop, \
         tc.tile_pool(name="osb", bufs=3) as osb, \
         tc.tile_pool(name="opsum", bufs=2, space="PSUM") as opsum:

        wo_sb = wop.tile([128, 4, D], F32R, tag="wo")
        for cc in range(4):
            raw = woraw.tile([128, D], FP32, tag="woraw")
            nc.sync.dma_start(out=raw[:], in_=wo_d[cc * 128:(cc + 1) * 128, :])
            nc.vector.tensor_copy(out=wo_sb[:, cc, :], in_=raw[:])

        for tcg in range(TCN):
            po = opsum.tile([128, D], FP32, tag="po")
            for j2 in range(2):
                for cc in range(4):
                    nc.tensor.matmul(
                        po[:, j2 * 512:(j2 + 1) * 512],
                        lhsT=ctx_sb[:, cc, tcg * 128:(tcg + 1) * 128],
                        rhs=wo_sb[:, cc, j2 * 512:(j2 + 1) * 512],
                        start=(cc == 0), stop=(cc == 3))
            ot = osb.tile([128, D], FP32, tag="ot")
            nc.vector.tensor_copy(out=ot[:], in_=po[:])
            nc.sync.dma_start(
                out=out_d[tcg * 128:(tcg + 1) * 128, :], in_=ot[:])


def build():
    nc = bacc.Bacc("TRN2", target_bir_lowering=False, debug=False, num_devices=8)
    xt_d = nc.dram_tensor("xt", [D, T], FP32, kind="ExternalInput").ap()
    wq_d = nc.dram_tensor("wq", [D, 512], FP32, kind="ExternalInput").ap()
    wk_d = nc.dram_tensor("wk", [D, 512], FP32, kind="ExternalInput").ap()
    wv_d = nc.dram_tensor("wv", [D, 512], FP32, kind="ExternalInput").ap()
    wo_d = nc.dram_tensor("wout", [512, D], FP32, kind="ExternalInput").ap()
    out_d = nc.dram_tensor("out", [T, D], FP32, kind="ExternalOutput").ap()
    with tile.TileContext(nc) as tc:
        with ExitStack() as ctx:
            _body(ctx, nc, tc, xt_d, wq_d, wk_d, wv_d, wo_d, out_d)
    nc.compile()
    return nc


_nc = None


def _get_nc():
    global _nc
    if _nc is None:
        _nc = build()
    return _nc


def make_in_maps(x, Wqkv, Wout):
    in_maps = []
    for c in range(8):
        b, g = divmod(c, 2)
        cs = slice(g * 512, (g + 1) * 512)
        in_maps.append({
            "xt": np.ascontiguousarray(x[b].T),
            "wq": np.ascontiguousarray(Wqkv[:, 0 * D:1 * D][:, cs]),
            "wk": np.ascontiguousarray(Wqkv[:, 1 * D:2 * D][:, cs]),
            "wv": np.ascontiguousarray(Wqkv[:, 2 * D:3 * D][:, cs]),
            "wout": np.ascontiguousarray(Wout[cs, :]),
        })
    return in_maps


def kernel(x, Wqkv, Wout, _trace=False):
    nc = _get_nc()
    x = np.asarray(x, dtype=np.float32)
    Wqkv = np.asarray(Wqkv, dtype=np.float32)
    Wout = np.asarray(Wout, dtype=np.float32)
    in_maps = make_in_maps(x, Wqkv, Wout)
    kwargs = {}
    if _trace:
        kwargs["trace"] = True
    res = run_bass_kernel_spmd(nc, in_maps, core_ids=list(range(8)), **kwargs)
    outs = [res.results[c]["out"] for c in range(8)]
    out = np.stack([outs[2 * b] + outs[2 * b + 1] for b in range(4)])
    if _trace:
        kernel.last_result = res
    return out
